# revision 7
# baseline (speedup 1.0000x reference)
"""Trainium2 Bass kernel for AngularTerms: out[p, a*8+s] = 2*f1[p,s]*f2[p,a]*fcj[p].

Self-contained: hardcodes shapes for vectors12 (2, 2000000, 3) f32 -> (2000000, 64) f32.
Data-parallel over the pair axis P across 8 NeuronCores; no collectives.

Math (per pair p, with v0, v1 the two displacement vectors):
  d_i   = |v_i|
  x     = 0.95*dot(v0,v1)/(d0*d1) = cos(theta)   (clamp is a no-op for this data)
  theta = pi/2 - arctan(x/y),  y = sqrt(1-x^2)   (y>0 since theta in [0,pi])
  f1[s] = ((1+cos(theta-Z_s))/2)^32 = cos((theta-Z_s)/2)^64
        = exp(64*ln(sin(-a/2 + (3pi/4 - Z_s/2)))),  a = arctan(x/y)
  fcj   = (cos(pi*d0/7)*cos(pi*d1/7))^2 = qq
  2*f2[a]*fcj = exp(ln2 + ln(qq) - 2*(s01-2*ShfA_a)^2),  s01 = d0+d1
  out[p, a*8+s] = f1[s] * {2*f2[a]*fcj}

Engine split (per-pair element slots):
  DVE : dot/norm reductions, reciprocals, u-path in fp16 (2x_1p mode), and
        the 64-wide outer-product multiply for 6 of 8 ShfA columns in bf16
        (all operands 2-byte packed -> 2x_1p).
  ACT : Square/Sqrt/Sin/Arctan/Ln/Exp. The f2 exp is emitted at WIDTH 2
        along s ([p,n,8,2]); since f2 is s-independent the same tile feeds
        all four s-blocks of the output TT -> expansion cost 16 slots, not 64.
  POOL: G8 = -a/2 + ZC_s (tensor_tensor add) and the outer product for the
        remaining 2 ShfA columns -- work the other engines can't absorb.
Activation tables: {sqrt, trig(sin+arctan), ln+exp} -> 3 loads per group.
"""
import sys

sys.path.insert(0, "/opt/trn_rl_repo")

import numpy as np
import ml_dtypes  # noqa: F401  (bf16 numpy dtype)
from contextlib import ExitStack

import concourse.bass as bass
import concourse.tile as tile
from concourse import bacc, mybir
from concourse.bass_utils import run_bass_kernel_spmd

F32 = mybir.dt.float32
F16 = mybir.dt.float16
BF16 = mybir.dt.bfloat16
AL = mybir.AluOpType
AF = mybir.ActivationFunctionType

P_TOTAL = 2_000_000
NCORES = 8
P_CORE = P_TOTAL // NCORES      # 250,000
N = 196                          # pairs per partition per tile
T = 10                           # tiles per core
P_PAD = 128 * N * T              # 250,880
CUTOFF = 3.5

NGROUPS = 2                      # phase-groups per core (table-load batches)
CAR_BUFS = 1                     # car pool bufs (1 = groups serialize on car)
NPOOL = 2                        # ShfA columns whose outer product runs on Pool
G8_ON_POOL = True                # G8 = a*-0.5 + ZC on Pool (else DVE stt)
QQ_ON_ACT = False                # qq=Square(q) on ACT thrashes tables in P2
SQ_ON_ACT = True                 # input squares on ACT (else DVE TT)

SHFA = np.array([0.9, 1.225, 1.55, 1.875, 2.2, 2.525, 2.85, 3.175], np.float32)
SHFZ = np.array([0.19634954, 0.58904862, 0.9817477, 1.37444679,
                 1.76714587, 2.15984495, 2.55254403, 2.94524311], np.float32)

_CACHE: dict = {}


def _build_nc(N=N, T=T, ngroups=NGROUPS, car_bufs=CAR_BUFS, npool=NPOOL,
              g8_on_pool=G8_ON_POOL, qq_on_act=QQ_ON_ACT, sq_on_act=SQ_ON_ACT,
              out_bufs=2):
    P_PAD = 128 * N * T
    TILE_PAIRS = 128 * N
    assert T % ngroups == 0
    TG = T // ngroups
    nd = 8 - npool                   # columns on the DVE expanded path
    nc = bacc.Bacc()
    vec = nc.declare_dram_parameter("vectors12", [2, P_PAD, 3], F32, isOutput=False)
    cst = nc.declare_dram_parameter("cst", [128, 8], F32, isOutput=False)
    cst16 = nc.declare_dram_parameter("cst16", [128, 8 * N], F16, isOutput=False)
    out = nc.declare_dram_parameter("out", [P_PAD, 64], BF16, isOutput=True)

    with tile.TileContext(nc) as tc, ExitStack() as ctx:
        const = ctx.enter_context(tc.tile_pool(name="const", bufs=1))
        carp = ctx.enter_context(tc.tile_pool(name="car", bufs=car_bufs))
        pA = ctx.enter_context(tc.tile_pool(name="pA", bufs=2))
        sm = ctx.enter_context(tc.tile_pool(name="sm", bufs=2))
        pC = ctx.enter_context(tc.tile_pool(name="pC", bufs=2))
        outp = ctx.enter_context(tc.tile_pool(name="outp", bufs=out_bufs))

        cstT = const.tile([128, 8], F32)
        nc.sync.dma_start(cstT[:], cst[:])
        ZC = cstT[:, 0:8]            # 3pi/4 - ShfZ/2
        A2E = const.tile([128, 8 * N], F16)   # 2*ShfA_a replicated over n
        nc.sync.dma_start(A2E[:], cst16[:])

        def const_scalar(val, name):
            t = const.tile([128, 1], F32, tag=name)
            nc.vector.memset(t[:], float(val))
            return t[:]

        b_pi2 = const_scalar(np.pi / 2, "pi2")
        b_one = const_scalar(1.0, "one")
        b_ln2 = const_scalar(float(np.log(2.0)), "ln2")

        act = nc.scalar.activation

        for g in range(ngroups):
            # carried per-tile state: d16(2N f16) txy(N f32) cwq(9N f32:
            # CW 8N + qq N) usq16(8N f16)
            car_d = carp.tile([128, 2 * N * TG], F16, tag="car_d")
            car_txy = carp.tile([128, N * TG], F32, tag="car_txy")
            car_cwq = carp.tile([128, 9 * N * TG], F32, tag="car_cwq")
            car_usq = carp.tile([128, 8 * N * TG], F16, tag="car_usq")

            def sl(tile_ap, w, tl):
                return tile_ap[:, tl * w: (tl + 1) * w]

            # ---------------- Phase 1: sqrt table set --------------------
            for tl in range(TG):
                base = (g * TG + tl) * TILE_PAIRS
                VV = pA.tile([128, 6 * N], F32, tag="VV")
                nc.sync.dma_start(
                    VV[:, : 3 * N],
                    vec[0, base: base + TILE_PAIRS, :].rearrange("(p n) c -> p (n c)", p=128),
                )
                nc.sync.dma_start(
                    VV[:, 3 * N:],
                    vec[1, base: base + TILE_PAIRS, :].rearrange("(p n) c -> p (n c)", p=128),
                )
                # T9 = [v0*v1 (3N) | v0^2,v1^2 (6N)]
                T9 = pA.tile([128, 9 * N], F32, tag="T9")
                nc.vector.tensor_tensor(T9[:, :3 * N], VV[:, :3 * N], VV[:, 3 * N:], AL.mult)
                if sq_on_act:
                    act(T9[:, 3 * N:], VV[:], AF.Square)
                else:
                    nc.vector.tensor_tensor(T9[:, 3 * N:], VV[:], VV[:], AL.mult)
                # reduce c: RD = [dot (N) | d0^2 (N) | d1^2 (N)]
                R3 = T9[:].rearrange("p (g n c) -> p (g n) c", g=3, c=3)
                RD = pA.tile([128, 3 * N], F32, tag="RD")
                nc.vector.tensor_tensor(RD[:], R3[:, :, 0], R3[:, :, 1], AL.add)
                nc.vector.tensor_tensor(RD[:], RD[:], R3[:, :, 2], AL.add)

                d = pA.tile([128, 2 * N], F32, tag="d")
                act(d[:], RD[:, N:], AF.Sqrt)
                d16_sl = sl(car_d, 2 * N, tl)
                act(d16_sl, d[:], AF.Copy)           # carry d as fp16 for Sin

                m = sm.tile([128, N], F32, tag="m")
                nc.vector.tensor_tensor(m[:], d[:, :N], d[:, N:], AL.mult)
                rm = sm.tile([128, N], F32, tag="rm")
                nc.vector.reciprocal_approx_fast(rm[:], m[:])
                x = sm.tile([128, N], F32, tag="x")
                nc.vector.scalar_tensor_tensor(x[:], RD[:, :N], 0.95, rm[:], AL.mult, AL.mult)
                x2 = sm.tile([128, N], F32, tag="x2")
                act(x2[:], x[:], AF.Square)
                y = sm.tile([128, N], F32, tag="y")
                act(y[:], x2[:], AF.Sqrt, bias=b_one, scale=-1.0)
                ry = sm.tile([128, N], F32, tag="ry")
                nc.vector.reciprocal_approx_fast(ry[:], y[:])
                txy_sl = sl(car_txy, N, tl)
                nc.vector.tensor_tensor(txy_sl, x[:], ry[:], AL.mult)

                # s01 directly in fp16 from the carried d16 (2x_1p TT); the
                # u-path consumed an fp16 s01 anyway so precision is unchanged
                s01_16 = sm.tile([128, N], F16, tag="s01_16")
                nc.vector.tensor_tensor(s01_16[:], d16_sl[:, :N], d16_sl[:, N:], AL.add)
                # u-path in fp16, [a, n] layout (n innermost -> 2x_1p)
                u16 = sm.tile([128, 8 * N], F16, tag="u16")
                s01b = s01_16[:][:, None, :].to_broadcast([128, 8, N])
                A2v = A2E[:].rearrange("p (a n) -> p a n", a=8)
                u16v = u16[:].rearrange("p (a n) -> p a n", a=8)
                nc.vector.tensor_tensor(u16v, s01b, A2v, AL.subtract)
                usq_sl = sl(car_usq, 8 * N, tl)
                nc.vector.tensor_tensor(usq_sl, u16[:], u16[:], AL.mult)

            # ---------------- Phase 2: trig table set --------------------
            for tl in range(TG):
                d16_sl = sl(car_d, 2 * N, tl)
                txy_sl = sl(car_txy, N, tl)
                cwq_sl = sl(car_cwq, 9 * N, tl)
                cw_sl = cwq_sl[:, : 8 * N]
                qq_sl = cwq_sl[:, 8 * N:]

                S12 = sm.tile([128, 2 * N], F32, tag="S12")
                act(S12[:], d16_sl, AF.Sin, bias=b_pi2, scale=float(-np.pi / 7))
                q = sm.tile([128, N], F32, tag="q")
                nc.vector.tensor_tensor(q[:], S12[:, :N], S12[:, N:], AL.mult)
                if qq_on_act:
                    act(qq_sl, q[:], AF.Square)
                else:
                    nc.vector.tensor_tensor(qq_sl, q[:], q[:], AL.mult)

                a8 = sm.tile([128, N], F32, tag="a8")
                act(a8[:], txy_sl, AF.Arctan)
                # G8 = -a/2 + ZC_s  -> into cw slot, then Sin in place
                cwv = cw_sl.rearrange("p (n s) -> p n s", s=8)
                ZCb = ZC[:, None, :].to_broadcast([128, N, 8])
                if g8_on_pool:
                    ah = sm.tile([128, N], F32, tag="ah")
                    nc.vector.tensor_scalar_mul(ah[:], a8[:], -0.5)
                    ahb = ah[:][:, :, None].to_broadcast([128, N, 8])
                    nc.gpsimd.tensor_tensor(cwv, ahb, ZCb, AL.add)
                else:
                    a8b = a8[:][:, :, None].to_broadcast([128, N, 8])
                    nc.vector.scalar_tensor_tensor(cwv, a8b, -0.5, ZCb, AL.mult, AL.add)
                act(cw_sl, cw_sl, AF.Sin)

            # ---------------- Phase 3: ln+exp table set ------------------
            for tl in range(TG):
                base = (g * TG + tl) * TILE_PAIRS
                cwq_sl = sl(car_cwq, 9 * N, tl)
                cw_sl = cwq_sl[:, : 8 * N]
                qq_sl = cwq_sl[:, 8 * N:]
                usq_sl = sl(car_usq, 8 * N, tl)

                act(cw_sl, cw_sl, AF.Ln)             # lnC in place
                lnqq16 = sm.tile([128, N], F16, tag="lnqq16")
                act(lnqq16[:], qq_sl, AF.Ln)
                F1 = pC.tile([128, 8 * N], BF16, tag="F1")
                act(F1[:], cw_sl, AF.Exp, scale=64.0)

                # W2 = 2*u^2 - ln(qq)  (fp16, [a, n] layout, in place on usq)
                usqv = usq_sl.rearrange("p (a n) -> p a n", a=8)
                lnqb = lnqq16[:][:, None, :].to_broadcast([128, 8, N])
                nc.vector.scalar_tensor_tensor(usqv, usqv, 2.0, lnqb,
                                               AL.mult, AL.subtract)

                # F2E[p, n, a, 2] = exp(-W2 + ln2), width-2 along s
                F2E = pC.tile([128, 16 * N], BF16, tag="F2E")
                F2Ev = F2E[:].rearrange("p (n a s) -> p n a s", a=8, s=2)
                W2t = usq_sl.rearrange("p (a n) -> p n a", a=8)
                W2b = W2t[:, :, :, None].to_broadcast([128, N, 8, 2])
                act(F2Ev, W2b, AF.Exp, bias=b_ln2, scale=-1.0)

                OUT = outp.tile([128, 64 * N], BF16, tag="OUT")
                OUTv = OUT[:].rearrange("p (n a s) -> p n a s", a=8, s=8)
                F1v = F1[:].rearrange("p (n s) -> p n s", s=8)

                # DVE expanded path: columns [0, nd), four s-blocks of width 2
                for sb in range(4):
                    ss = slice(2 * sb, 2 * sb + 2)
                    F1b = F1v[:, :, ss][:, :, None, :].to_broadcast([128, N, nd, 2])
                    nc.vector.tensor_tensor(OUTv[:, :, :nd, ss], F1b,
                                            F2Ev[:, :, :nd, :], AL.mult)
                # Pool path: columns [nd, 8), full s width
                if npool:
                    F1pb = F1v[:, :, None, :].to_broadcast([128, N, npool, 8])
                    F2nb = F2Ev[:, :, nd:, 0:1].to_broadcast([128, N, npool, 8])
                    nc.gpsimd.tensor_tensor(OUTv[:, :, nd:, :], F1pb, F2nb, AL.mult)

                nc.sync.dma_start(
                    out[base: base + TILE_PAIRS, :].rearrange("(p n) f -> p (n f)", p=128),
                    OUT[:],
                )

    # Restrict activation-table membership so each phase's functions resolve
    # to one set (avoids the greedy first-set binding thrashing table loads).
    import concourse.bacc as bacc_mod
    from concourse.hw_specs import get_activation_tables as _real_gat
    keep = {"sqrt_and_others", "trig_and_small", "natural_log_exp_and_others"}

    def _gat(arch):
        return {k: (v if k in keep else set()) for k, v in _real_gat(arch).items()}

    bacc_mod.get_activation_tables = _gat
    try:
        nc.compile()
    finally:
        bacc_mod.get_activation_tables = _real_gat
    return nc


def _cst_arrays(N=N):
    zc = (0.75 * np.pi - 0.5 * SHFZ).astype(np.float32)
    cst = np.broadcast_to(zc, (128, 8)).copy()
    a2 = np.repeat((2.0 * SHFA).astype(np.float16), N)      # [a, n] layout
    cst16 = np.broadcast_to(a2, (128, 8 * N)).copy()
    return cst, cst16


def _run(vectors12: np.ndarray, trace: bool = False):
    if "nc" not in _CACHE:
        _CACHE["nc"] = _build_nc()
    nc = _CACHE["nc"]

    v = np.ascontiguousarray(np.asarray(vectors12, dtype=np.float32))
    pad = np.zeros((2, P_PAD - P_CORE, 3), np.float32)
    pad[:, :, 0] = 1.0  # unit vectors: all downstream math well-defined
    cst, cst16 = _cst_arrays()

    in_maps = []
    for i in range(NCORES):
        shard = v[:, i * P_CORE: (i + 1) * P_CORE, :]
        shard = np.concatenate([shard, pad], axis=1)
        in_maps.append({"vectors12": np.ascontiguousarray(shard),
                        "cst": cst, "cst16": cst16})

    res = run_bass_kernel_spmd(nc, in_maps, core_ids=list(range(NCORES)),
                               trace=trace)
    out = np.empty((P_TOTAL, 64), np.float32)
    for i in range(NCORES):
        shard_out = np.asarray(res.results[i]["out"])[:P_CORE]
        out[i * P_CORE: (i + 1) * P_CORE] = shard_out.astype(np.float32)
    return out, res


def kernel(vectors12, EtaA=None, Zeta=None, ShfA=None, ShfZ=None):
    out, _ = _run(vectors12, trace=False)
    return out


# revision 8
# speedup vs baseline: 1.1984x; 1.1984x over previous
"""Trainium2 Bass kernel for AngularTerms: out[p, a*8+s] = 2*f1[p,s]*f2[p,a]*fcj[p].

Self-contained: hardcodes shapes for vectors12 (2, 2000000, 3) f32 -> (2000000, 64) f32.
Data-parallel over the pair axis P across 8 NeuronCores; no collectives.

Math (per pair p, with v0, v1 the two displacement vectors; d_i = |v_i|):
  x     = 0.95*dot(v0,v1)/(d0*d1) = cos(theta)
  theta = pi/2 - arctan(x/y),  y = sqrt(1-x^2)   (y>0 since theta in [0,pi])
  f1[s] = ((1+cos(theta-Z_s))/2)^32 = exp(64*ln(sin(-a/2 + (3pi/4 - Z_s/2))))
  fcj   = (cos(pi*d0/7)*cos(pi*d1/7))^2 = qq
  2*f2[a]*fcj = exp(ln2 + ln(qq) - 2*(s01-2*ShfA_a)^2),  s01 = d0+d1
  out[p, a*8+s] = f1[s] * {2*f2[a]*fcj}

sqrt(2) is folded into d (free via the Sqrt activation's scale): then
s01' = sqrt2*s01 and u' = s01' - 2*sqrt2*ShfA gives u'^2 = 2u^2 directly,
so the whole u-path runs in fp16 tensor_tensor at DVE 2x_1p with no
scalar_tensor_tensor (whose fp16 path measured 2x slower than TT).

Engine notes (hardware-measured):
  - DVE 2x_1p needs ALL operands 2-byte with innermost stride +-1; innermost
    runs of 2 cost ~1.6ns/el (vs 0.45 at runs of 8) -> expansion is emitted
    full-width-8 by ACT exp, never width-2.
  - GPSIMD shares the DVE SBUF port: offloading work there poisoned DVE
    throughput (PR 740->2120ns). Pool is left idle on purpose.
  - ACT instruction overhead ~0.5us -> small activations are merged/moved
    to DVE where possible.
Activation tables: {sqrt, trig(sin+arctan), ln+exp}; the ACT stream is
phase-barriered so each group loads each table exactly once.
"""
import sys

sys.path.insert(0, "/opt/trn_rl_repo")

import numpy as np
import ml_dtypes  # noqa: F401  (bf16 numpy dtype)
from contextlib import ExitStack

import concourse.bass as bass
import concourse.tile as tile
from concourse import bacc, mybir
from concourse.bass_utils import run_bass_kernel_spmd

F32 = mybir.dt.float32
F16 = mybir.dt.float16
BF16 = mybir.dt.bfloat16
AL = mybir.AluOpType
AF = mybir.ActivationFunctionType

P_TOTAL = 2_000_000
NCORES = 8
P_CORE = P_TOTAL // NCORES      # 250,000
N = 196                          # pairs per partition per tile
T = 10                           # tiles per core
P_PAD = 128 * N * T              # 250,880
CUTOFF = 3.5
SQ2 = float(np.sqrt(2.0))

NGROUPS = 2                      # phase-groups per core (table-load batches)
NE = 7                           # ShfA columns via full-width ACT exp-expand
F2REP_BUFS = 1
USE_BARRIERS = True              # hard ACT phase barriers (kill table thrash)
USE_REDUCE = True                # c-axis sums via tensor_reduce (not strided TT)

SHFA = np.array([0.9, 1.225, 1.55, 1.875, 2.2, 2.525, 2.85, 3.175], np.float32)
SHFZ = np.array([0.19634954, 0.58904862, 0.9817477, 1.37444679,
                 1.76714587, 2.15984495, 2.55254403, 2.94524311], np.float32)

_CACHE: dict = {}


def _build_nc(N=N, T=T, ngroups=NGROUPS, ne=NE, use_barriers=USE_BARRIERS,
              use_reduce=USE_REDUCE, f2rep_bufs=F2REP_BUFS, out_bufs=2):
    P_PAD = 128 * N * T
    TILE_PAIRS = 128 * N
    assert T % ngroups == 0
    TG = T // ngroups
    nd = 8 - ne
    nc = bacc.Bacc()
    vec = nc.declare_dram_parameter("vectors12", [2, P_PAD, 3], F32, isOutput=False)
    cst = nc.declare_dram_parameter("cst", [128, 8], F32, isOutput=False)
    cst16 = nc.declare_dram_parameter("cst16", [128, 8 * N], F16, isOutput=False)
    out = nc.declare_dram_parameter("out", [P_PAD, 64], BF16, isOutput=True)

    from concourse.bass import _add_dep_helper
    phase_acts: list = []
    prev_marker = [None]

    def act(*args, **kw):
        ins = nc.scalar.activation(*args, **kw)
        if prev_marker[0] is not None:
            _add_dep_helper(ins.ins, prev_marker[0].ins, sync=False,
                            reason="act phase fan-out")
        phase_acts.append(ins)
        return ins

    with tile.TileContext(nc) as tc, ExitStack() as ctx:
        const = ctx.enter_context(tc.tile_pool(name="const", bufs=1))
        carp = ctx.enter_context(tc.tile_pool(name="car", bufs=1))
        pA = ctx.enter_context(tc.tile_pool(name="pA", bufs=2))
        sm = ctx.enter_context(tc.tile_pool(name="sm", bufs=2))
        pC = ctx.enter_context(tc.tile_pool(name="pC", bufs=2))
        pR = ctx.enter_context(tc.tile_pool(name="pR", bufs=f2rep_bufs))
        outp = ctx.enter_context(tc.tile_pool(name="outp", bufs=out_bufs))

        cstT = const.tile([128, 8], F32)
        nc.sync.dma_start(cstT[:], cst[:])
        ZC = cstT[:, 0:8]            # 3pi/4 - ShfZ/2
        A2E = const.tile([128, 8 * N], F16)   # 2*sqrt2*ShfA_a replicated over n
        nc.sync.dma_start(A2E[:], cst16[:])

        def const_scalar(val, name):
            t = const.tile([128, 1], F32, tag=name)
            nc.vector.memset(t[:], float(val))
            return t[:]

        b_pi2 = const_scalar(np.pi / 2, "pi2")
        b_one = const_scalar(1.0, "one")
        b_ln2 = const_scalar(float(np.log(2.0)), "ln2")
        dummy = const.tile([128, 1], F32, tag="dummy")
        nc.vector.memset(dummy[:], 0.0)

        def phase_barrier():
            if not use_barriers:
                return
            marker = nc.scalar.activation(dummy[:], dummy[:], AF.Copy)
            for a in phase_acts:
                _add_dep_helper(marker.ins, a.ins, sync=False,
                                reason="act phase fan-in")
            phase_acts.clear()
            prev_marker[0] = marker

        for g in range(ngroups):
            # carried per-tile state
            car_d = carp.tile([128, 2 * N * TG], F32, tag="car_d")     # sqrt2*d
            car_txy = carp.tile([128, N * TG], F32, tag="car_txy")     # x/y
            car_cwq = carp.tile([128, 9 * N * TG], F32, tag="car_cwq")  # CW|qq
            car_usq = carp.tile([128, 8 * N * TG], F16, tag="car_usq")  # 2u^2

            def sl(tile_ap, w, tl):
                return tile_ap[:, tl * w: (tl + 1) * w]

            # ---------------- Phase 1: sqrt table set --------------------
            for tl in range(TG):
                base = (g * TG + tl) * TILE_PAIRS
                VV = pA.tile([128, 6 * N], F32, tag="VV")
                nc.sync.dma_start(
                    VV[:, : 3 * N],
                    vec[0, base: base + TILE_PAIRS, :].rearrange("(p n) c -> p (n c)", p=128),
                )
                nc.sync.dma_start(
                    VV[:, 3 * N:],
                    vec[1, base: base + TILE_PAIRS, :].rearrange("(p n) c -> p (n c)", p=128),
                )
                PR = pA.tile([128, 3 * N], F32, tag="PR")
                nc.vector.tensor_tensor(PR[:], VV[:, :3 * N], VV[:, 3 * N:], AL.mult)
                act(VV[:], VV[:], AF.Square)          # in place; anti-dep on PR
                RD = pA.tile([128, 3 * N], F32, tag="RD")
                if use_reduce:
                    nc.vector.tensor_reduce(
                        RD[:, :N], PR[:].rearrange("p (n c) -> p n c", c=3),
                        mybir.AxisListType.X, AL.add)
                    nc.vector.tensor_reduce(
                        RD[:, N:], VV[:].rearrange("p (i n c) -> p (i n) c", i=2, c=3),
                        mybir.AxisListType.X, AL.add)
                else:
                    T9v = None  # unused
                    R3p = PR[:].rearrange("p (n c) -> p n c", c=3)
                    R3s = VV[:].rearrange("p (i n c) -> p (i n) c", i=2, c=3)
                    nc.vector.tensor_tensor(RD[:, :N], R3p[:, :, 0], R3p[:, :, 1], AL.add)
                    nc.vector.tensor_tensor(RD[:, :N], RD[:, :N], R3p[:, :, 2], AL.add)
                    nc.vector.tensor_tensor(RD[:, N:], R3s[:, :, 0], R3s[:, :, 1], AL.add)
                    nc.vector.tensor_tensor(RD[:, N:], RD[:, N:], R3s[:, :, 2], AL.add)

                d_sl = sl(car_d, 2 * N, tl)
                act(d_sl, RD[:, N:], AF.Sqrt, scale=2.0)   # sqrt2 * d

                # my = [2*d0*d1 | y];  rmy = 1/my
                my = sm.tile([128, 2 * N], F32, tag="my")
                nc.vector.tensor_tensor(my[:, :N], d_sl[:, :N], d_sl[:, N:], AL.mult)
                x = sm.tile([128, N], F32, tag="x")
                cc = sm.tile([128, N], F32, tag="cc")
                rmy = sm.tile([128, 2 * N], F32, tag="rmy")
                nc.vector.reciprocal_approx_fast(rmy[:, :N], my[:, :N])
                nc.vector.scalar_tensor_tensor(x[:], RD[:, :N], 1.9, rmy[:, :N],
                                               AL.mult, AL.mult)
                nc.vector.scalar_tensor_tensor(cc[:], x[:], -1.0, x[:],
                                               AL.mult, AL.mult)
                act(my[:, N:], cc[:], AF.Sqrt, bias=b_one)   # y = sqrt(1-x^2)
                nc.vector.reciprocal_approx_fast(rmy[:, N:], my[:, N:])
                txy_sl = sl(car_txy, N, tl)
                nc.vector.tensor_tensor(txy_sl, x[:], rmy[:, N:], AL.mult)

                # u-path (all fp16 2x_1p):  s01' = sqrt2*(d0+d1)
                s01_16 = sm.tile([128, N], F16, tag="s01_16")
                nc.vector.tensor_tensor(s01_16[:], d_sl[:, :N], d_sl[:, N:], AL.add)
                u16 = sm.tile([128, 8 * N], F16, tag="u16")
                s01b = s01_16[:][:, None, :].to_broadcast([128, 8, N])
                A2v = A2E[:].rearrange("p (a n) -> p a n", a=8)
                u16v = u16[:].rearrange("p (a n) -> p a n", a=8)
                nc.vector.tensor_tensor(u16v, s01b, A2v, AL.subtract)
                usq_sl = sl(car_usq, 8 * N, tl)
                nc.vector.tensor_tensor(usq_sl, u16[:], u16[:], AL.mult)  # 2u^2

            phase_barrier()

            # ---------------- Phase 2: trig table set --------------------
            for tl in range(TG):
                d_sl = sl(car_d, 2 * N, tl)
                txy_sl = sl(car_txy, N, tl)
                cwq_sl = sl(car_cwq, 9 * N, tl)
                cw_sl = cwq_sl[:, : 8 * N]
                qq_sl = cwq_sl[:, 8 * N:]

                S12 = sm.tile([128, 2 * N], F32, tag="S12")
                act(S12[:], d_sl, AF.Sin, bias=b_pi2, scale=float(-np.pi / 7 / SQ2))
                q = sm.tile([128, N], F32, tag="q")
                nc.vector.tensor_tensor(q[:], S12[:, :N], S12[:, N:], AL.mult)
                nc.vector.tensor_tensor(qq_sl, q[:], q[:], AL.mult)

                a8 = sm.tile([128, N], F32, tag="a8")
                act(a8[:], txy_sl, AF.Arctan)
                # G8 = -a/2 + ZC_s  -> into cw slot, then Sin in place
                cwv = cw_sl.rearrange("p (n s) -> p n s", s=8)
                ZCb = ZC[:, None, :].to_broadcast([128, N, 8])
                a8b = a8[:][:, :, None].to_broadcast([128, N, 8])
                nc.vector.scalar_tensor_tensor(cwv, a8b, -0.5, ZCb, AL.mult, AL.add)
                act(cw_sl, cw_sl, AF.Sin)

            phase_barrier()

            # ---------------- Phase 3: ln+exp table set ------------------
            for tl in range(TG):
                base = (g * TG + tl) * TILE_PAIRS
                cwq_sl = sl(car_cwq, 9 * N, tl)
                cw_sl = cwq_sl[:, : 8 * N]
                qq_sl = cwq_sl[:, 8 * N:]
                usq_sl = sl(car_usq, 8 * N, tl)

                act(cw_sl, cw_sl, AF.Ln)             # lnC in place
                lnqq16 = sm.tile([128, N], F16, tag="lnqq16")
                act(lnqq16[:], qq_sl, AF.Ln)
                F1 = pC.tile([128, 8 * N], BF16, tag="F1")
                act(F1[:], cw_sl, AF.Exp, scale=64.0)

                # W2 = 2u^2 - ln(qq)   (fp16 TT 2x_1p, in place on usq)
                usqv = usq_sl.rearrange("p (a n) -> p a n", a=8)
                lnqb = lnqq16[:][:, None, :].to_broadcast([128, 8, N])
                nc.vector.tensor_tensor(usqv, usqv, lnqb, AL.subtract)

                # F2Q = exp(-W2 + ln2): full-width-8 for ne cols, narrow rest
                W2t = usq_sl.rearrange("p (a n) -> p n a", a=8)
                F2R = pR.tile([128, ne * 8 * N], BF16, tag="F2R")
                F2Rv = F2R[:].rearrange("p (n a s) -> p n a s", a=ne, s=8)
                W2be = W2t[:, :, :ne, None].to_broadcast([128, N, ne, 8])
                act(F2Rv, W2be, AF.Exp, bias=b_ln2, scale=-1.0)

                OUT = outp.tile([128, 64 * N], BF16, tag="OUT")
                OUTv = OUT[:].rearrange("p (n a s) -> p n a s", a=8, s=8)
                F1v = F1[:].rearrange("p (n s) -> p n s", s=8)

                F1be = F1v[:, :, None, :].to_broadcast([128, N, ne, 8])
                nc.vector.tensor_tensor(OUTv[:, :, :ne, :], F1be, F2Rv, AL.mult)

                if nd:
                    F2n = sm.tile([128, nd * N], BF16, tag="F2n")
                    F2nv = F2n[:].rearrange("p (a n) -> p a n", a=nd)
                    act(F2nv, usqv[:, ne:, :], AF.Exp, bias=b_ln2, scale=-1.0)
                    F1bd = F1v[:, :, None, :].to_broadcast([128, N, nd, 8])
                    F2nb = F2nv.rearrange("p a n -> p n a")[:, :, :, None] \
                        .to_broadcast([128, N, nd, 8])
                    nc.vector.tensor_tensor(OUTv[:, :, ne:, :], F1bd, F2nb, AL.mult)

                nc.sync.dma_start(
                    out[base: base + TILE_PAIRS, :].rearrange("(p n) f -> p (n f)", p=128),
                    OUT[:],
                )

            if g + 1 < ngroups:
                phase_barrier()

    # Restrict activation-table membership so each phase's functions resolve
    # to one set (avoids the greedy first-set binding thrashing table loads).
    import concourse.bacc as bacc_mod
    from concourse.hw_specs import get_activation_tables as _real_gat
    keep = {"sqrt_and_others", "trig_and_small", "natural_log_exp_and_others"}

    def _gat(arch):
        return {k: (v if k in keep else set()) for k, v in _real_gat(arch).items()}

    bacc_mod.get_activation_tables = _gat
    try:
        nc.compile()
    finally:
        bacc_mod.get_activation_tables = _real_gat
    return nc


def _cst_arrays(N=N):
    zc = (0.75 * np.pi - 0.5 * SHFZ).astype(np.float32)
    cst = np.broadcast_to(zc, (128, 8)).copy()
    a2 = np.repeat((2.0 * SQ2 * SHFA).astype(np.float16), N)   # [a, n] layout
    cst16 = np.broadcast_to(a2, (128, 8 * N)).copy()
    return cst, cst16


def _run(vectors12: np.ndarray, trace: bool = False):
    if "nc" not in _CACHE:
        _CACHE["nc"] = _build_nc()
    nc = _CACHE["nc"]

    v = np.ascontiguousarray(np.asarray(vectors12, dtype=np.float32))
    pad = np.zeros((2, P_PAD - P_CORE, 3), np.float32)
    pad[:, :, 0] = 1.0  # unit vectors: all downstream math well-defined
    cst, cst16 = _cst_arrays()

    in_maps = []
    for i in range(NCORES):
        shard = v[:, i * P_CORE: (i + 1) * P_CORE, :]
        shard = np.concatenate([shard, pad], axis=1)
        in_maps.append({"vectors12": np.ascontiguousarray(shard),
                        "cst": cst, "cst16": cst16})

    res = run_bass_kernel_spmd(nc, in_maps, core_ids=list(range(NCORES)),
                               trace=trace)
    out = np.empty((P_TOTAL, 64), np.float32)
    for i in range(NCORES):
        shard_out = np.asarray(res.results[i]["out"])[:P_CORE]
        out[i * P_CORE: (i + 1) * P_CORE] = shard_out.astype(np.float32)
    return out, res


def kernel(vectors12, EtaA=None, Zeta=None, ShfA=None, ShfZ=None):
    out, _ = _run(vectors12, trace=False)
    return out


# revision 13
# speedup vs baseline: 1.3479x; 1.1247x over previous
"""Trainium2 Bass kernel for AngularTerms: out[p, a*8+s] = 2*f1[p,s]*f2[p,a]*fcj[p].

Self-contained: hardcodes shapes for vectors12 (2, 2000000, 3) f32 -> (2000000, 64) f32.
Data-parallel over the pair axis P across 8 NeuronCores; no collectives.

Math (per pair p, with v0, v1 the two displacement vectors; d_i = |v_i|):
  x     = 0.95*dot(v0,v1)/(d0*d1) = cos(theta)
  theta = pi/2 - arctan(x/y),  y = sqrt(1-x^2)   (y>0 since theta in [0,pi])
  f1[s] = ((1+cos(theta-Z_s))/2)^32 = exp(64*ln(sin(-a/2 + (3pi/4 - Z_s/2))))
  fcj   = (cos(pi*d0/7)*cos(pi*d1/7))^2 = qq
  2*f2[a]*fcj = exp(ln2 + ln(qq) - 2*(s01-2*ShfA_a)^2),  s01 = d0+d1
  out[p, a*8+s] = f1[s] * {2*f2[a]*fcj}

sqrt(2) is folded into d (free via the Sqrt activation's scale): then
s01' = sqrt2*s01 and u' = s01' - 2*sqrt2*ShfA gives u'^2 = 2u^2 directly,
so the whole u-path runs in fp16 tensor_tensor at DVE 2x_1p.

Schedule: three activation-table phases per group ({sqrt} {sin,arctan}
{ln,exp}), ACT stream hard-ordered by barrier markers, and groups emitted
SKEWED -- P1(0) P2(0) P1(1) P3(0) P2(1) P3(1) -- so the DVE-heavy P1 of the
next group executes under the ACT-heavy P3 (exp expansion) of the previous
one. Table loads stay at 3 per group. The u-path runs inside P3 (DVE) so
only s01 (fp16, double-buffered) crosses the P1->P3 skew boundary.

Hardware-measured notes: DVE 2x_1p needs all operands 2-byte innermost
stride +-1 (broadcast middle dims fine); innermost runs of 2 are ~3x slower
than runs of 8; GPSIMD shares the DVE SBUF port and poisons DVE throughput
(kept idle); fp16 scalar_tensor_tensor has no fast uop (plain TT used).
"""
import sys

sys.path.insert(0, "/opt/trn_rl_repo")

import numpy as np
import ml_dtypes  # noqa: F401  (bf16 numpy dtype)
from contextlib import ExitStack

import concourse.bass as bass
import concourse.tile as tile
from concourse import bacc, mybir
from concourse.bass_utils import run_bass_kernel_spmd

F32 = mybir.dt.float32
F16 = mybir.dt.float16
BF16 = mybir.dt.bfloat16
AL = mybir.AluOpType
AF = mybir.ActivationFunctionType

P_TOTAL = 2_000_000
NCORES = 8
P_CORE = P_TOTAL // NCORES      # 250,000
N = 196                          # pairs per partition per tile
T = 10                           # tiles per core
P_PAD = 128 * N * T              # 250,880
SQ2 = float(np.sqrt(2.0))

NGROUPS = 2                      # phase-groups per core (table-load batches)
NE = 7                           # ShfA columns via full-width ACT exp-expand
F2REP_BUFS = 2
USE_BARRIERS = True              # hard ACT phase barriers (kill table thrash)
SKEW = True                      # pipeline groups: P1(g+1) before P3(g)

SHFA = np.array([0.9, 1.225, 1.55, 1.875, 2.2, 2.525, 2.85, 3.175], np.float32)
SHFZ = np.array([0.19634954, 0.58904862, 0.9817477, 1.37444679,
                 1.76714587, 2.15984495, 2.55254403, 2.94524311], np.float32)

_CACHE: dict = {}


def _build_nc(N=N, T=T, ngroups=NGROUPS, ne=NE, use_barriers=USE_BARRIERS,
              f2rep_bufs=F2REP_BUFS, out_bufs=2, skew=SKEW):
    P_PAD = 128 * N * T
    TILE_PAIRS = 128 * N
    assert T % ngroups == 0
    TG = T // ngroups
    nd = 8 - ne
    nc = bacc.Bacc()
    vec = nc.declare_dram_parameter("vectors12", [2, P_PAD, 3], F32, isOutput=False)
    cst = nc.declare_dram_parameter("cst", [128, 8], F32, isOutput=False)
    cst16 = nc.declare_dram_parameter("cst16", [128, 8 * N], F16, isOutput=False)
    out = nc.declare_dram_parameter("out", [P_PAD, 64], BF16, isOutput=True)

    from concourse.bass import _add_dep_helper
    phase_acts: list = []
    prev_marker = [None]

    def act(*args, **kw):
        ins = nc.scalar.activation(*args, **kw)
        if prev_marker[0] is not None:
            _add_dep_helper(ins.ins, prev_marker[0].ins, sync=False,
                            reason="act phase fan-out")
        phase_acts.append(ins)
        return ins

    with tile.TileContext(nc) as tc, ExitStack() as ctx:
        const = ctx.enter_context(tc.tile_pool(name="const", bufs=1))
        carp = ctx.enter_context(tc.tile_pool(name="car", bufs=1))
        car2 = ctx.enter_context(tc.tile_pool(name="car2", bufs=2))
        pA = ctx.enter_context(tc.tile_pool(name="pA", bufs=2))
        sm = ctx.enter_context(tc.tile_pool(name="sm", bufs=2))
        pC = ctx.enter_context(tc.tile_pool(name="pC", bufs=2))
        pR = ctx.enter_context(tc.tile_pool(name="pR", bufs=f2rep_bufs))
        outp = ctx.enter_context(tc.tile_pool(name="outp", bufs=out_bufs))

        cstT = const.tile([128, 8], F32)
        nc.sync.dma_start(cstT[:], cst[:])
        ZC = cstT[:, 0:8]            # 3pi/4 - ShfZ/2
        A2E = const.tile([128, 8 * N], F16)   # 2*sqrt2*ShfA_a replicated over n
        nc.sync.dma_start(A2E[:], cst16[:])

        def const_scalar(val, name):
            t = const.tile([128, 1], F32, tag=name)
            nc.vector.memset(t[:], float(val))
            return t[:]

        b_pi2 = const_scalar(np.pi / 2, "pi2")
        b_one = const_scalar(1.0, "one")
        b_ln2 = const_scalar(float(np.log(2.0)), "ln2")
        dummy = const.tile([128, 1], F32, tag="dummy")
        nc.vector.memset(dummy[:], 0.0)

        def phase_barrier():
            if not use_barriers:
                return
            marker = nc.scalar.activation(dummy[:], dummy[:], AF.Copy)
            for a in phase_acts:
                _add_dep_helper(marker.ins, a.ins, sync=False,
                                reason="act phase fan-in")
            phase_acts.clear()
            prev_marker[0] = marker

        # per-group carried tiles (bufs=1 tags are shared across groups; the
        # skew keeps their write-after-read ordering legal; s01 crosses the
        # P1(g+1)-before-P3(g) boundary so it is double-buffered in car2)
        cars = {}
        for g in range(ngroups):
            car_d = carp.tile([128, 2 * N * TG], F32, tag="car_d")
            car_txy = carp.tile([128, N * TG], F32, tag="car_txy")
            car_cw = carp.tile([128, 8 * N * TG], F32, tag="car_cw")
            car_qq = carp.tile([128, N * TG], F16, tag="car_qq")
            car_s01 = car2.tile([128, N * TG], F16, tag="car_s01")
            cars[g] = dict(d=car_d, txy=car_txy, cw=car_cw, qq=car_qq,
                           s01=car_s01)

        def sl(tile_ap, w, tl):
            return tile_ap[:, tl * w: (tl + 1) * w]

        def phase1(g):
            car = cars[g]
            for tl in range(TG):
                base = (g * TG + tl) * TILE_PAIRS
                VV = pA.tile([128, 6 * N], F32, tag="VV")
                nc.sync.dma_start(
                    VV[:, : 3 * N],
                    vec[0, base: base + TILE_PAIRS, :].rearrange("(p n) c -> p (n c)", p=128),
                )
                nc.sync.dma_start(
                    VV[:, 3 * N:],
                    vec[1, base: base + TILE_PAIRS, :].rearrange("(p n) c -> p (n c)", p=128),
                )
                PR = pA.tile([128, 3 * N], F32, tag="PR")
                nc.vector.tensor_tensor(PR[:], VV[:, :3 * N], VV[:, 3 * N:], AL.mult)
                act(VV[:], VV[:], AF.Square)          # in place; anti-dep on PR
                RD = pA.tile([128, 3 * N], F32, tag="RD")
                nc.vector.tensor_reduce(
                    RD[:, :N], PR[:].rearrange("p (n c) -> p n c", c=3),
                    mybir.AxisListType.X, AL.add)
                nc.vector.tensor_reduce(
                    RD[:, N:], VV[:].rearrange("p (i n c) -> p (i n) c", i=2, c=3),
                    mybir.AxisListType.X, AL.add)

                d_sl = sl(car['d'], 2 * N, tl)
                act(d_sl, RD[:, N:], AF.Sqrt, scale=2.0)   # sqrt2 * d

                my = sm.tile([128, 2 * N], F32, tag="my")   # [2*d0*d1 | y]
                nc.vector.tensor_tensor(my[:, :N], d_sl[:, :N], d_sl[:, N:], AL.mult)
                x = sm.tile([128, N], F32, tag="x")
                cc = sm.tile([128, N], F32, tag="cc")
                rmy = sm.tile([128, 2 * N], F32, tag="rmy")
                nc.vector.reciprocal_approx_fast(rmy[:, :N], my[:, :N])
                nc.vector.scalar_tensor_tensor(x[:], RD[:, :N], 1.9, rmy[:, :N],
                                               AL.mult, AL.mult)
                nc.vector.scalar_tensor_tensor(cc[:], x[:], -1.0, x[:],
                                               AL.mult, AL.mult)
                act(my[:, N:], cc[:], AF.Sqrt, bias=b_one)   # y = sqrt(1-x^2)
                nc.vector.reciprocal_approx_fast(rmy[:, N:], my[:, N:])
                nc.vector.tensor_tensor(sl(car['txy'], N, tl), x[:], rmy[:, N:], AL.mult)
                # s01' = sqrt2*(d0+d1), fp16 (sole P1 product the skewed P3 uses)
                nc.vector.tensor_tensor(sl(car['s01'], N, tl),
                                        d_sl[:, :N], d_sl[:, N:], AL.add)
            phase_barrier()

        def phase2(g):
            car = cars[g]
            for tl in range(TG):
                d_sl = sl(car['d'], 2 * N, tl)
                cw_sl = sl(car['cw'], 8 * N, tl)
                S12 = sm.tile([128, 2 * N], F32, tag="S12")
                act(S12[:], d_sl, AF.Sin, bias=b_pi2, scale=float(-np.pi / 7 / SQ2))
                q = sm.tile([128, N], F32, tag="q")
                nc.vector.tensor_tensor(q[:], S12[:, :N], S12[:, N:], AL.mult)
                nc.vector.tensor_tensor(sl(car['qq'], N, tl), q[:], q[:], AL.mult)

                a8 = sm.tile([128, N], F32, tag="a8")
                act(a8[:], sl(car['txy'], N, tl), AF.Arctan)
                # G8 = -a/2 + ZC_s  -> into cw slot, then Sin in place
                cwv = cw_sl.rearrange("p (n s) -> p n s", s=8)
                ZCb = ZC[:, None, :].to_broadcast([128, N, 8])
                a8b = a8[:][:, :, None].to_broadcast([128, N, 8])
                nc.vector.scalar_tensor_tensor(cwv, a8b, -0.5, ZCb, AL.mult, AL.add)
                act(cw_sl, cw_sl, AF.Sin)
            phase_barrier()

        def phase3(g):
            car = cars[g]
            for tl in range(TG):
                base = (g * TG + tl) * TILE_PAIRS
                cw_sl = sl(car['cw'], 8 * N, tl)

                act(cw_sl, cw_sl, AF.Ln)             # lnC in place
                lnqq16 = sm.tile([128, N], F16, tag="lnqq16")
                act(lnqq16[:], sl(car['qq'], N, tl), AF.Ln)
                F1 = pC.tile([128, 8 * N], BF16, tag="F1")
                act(F1[:], cw_sl, AF.Exp, scale=64.0)

                # u-path (fp16 TT 2x_1p): u' = s01' - 2sqrt2*A; W2 = u'^2 - lnqq
                s01_sl = sl(car['s01'], N, tl)
                u16 = sm.tile([128, 8 * N], F16, tag="u16")
                s01b = s01_sl[:, None, :].to_broadcast([128, 8, N])
                A2v = A2E[:].rearrange("p (a n) -> p a n", a=8)
                u16v = u16[:].rearrange("p (a n) -> p a n", a=8)
                nc.vector.tensor_tensor(u16v, s01b, A2v, AL.subtract)
                W2 = sm.tile([128, 8 * N], F16, tag="W2")
                W2v = W2[:].rearrange("p (a n) -> p a n", a=8)
                nc.vector.tensor_tensor(W2[:], u16[:], u16[:], AL.mult)
                lnqb = lnqq16[:][:, None, :].to_broadcast([128, 8, N])
                nc.vector.tensor_tensor(W2v, W2v, lnqb, AL.subtract)

                # F2Q = exp(-W2 + ln2): full-width-8 for ne cols, narrow rest
                W2t = W2[:].rearrange("p (a n) -> p n a", a=8)
                F2R = pR.tile([128, ne * 8 * N], BF16, tag="F2R")
                F2Rv = F2R[:].rearrange("p (n a s) -> p n a s", a=ne, s=8)
                W2be = W2t[:, :, :ne, None].to_broadcast([128, N, ne, 8])
                act(F2Rv, W2be, AF.Exp, bias=b_ln2, scale=-1.0)

                OUT = outp.tile([128, 64 * N], BF16, tag="OUT")
                OUTv = OUT[:].rearrange("p (n a s) -> p n a s", a=8, s=8)
                F1v = F1[:].rearrange("p (n s) -> p n s", s=8)

                F1be = F1v[:, :, None, :].to_broadcast([128, N, ne, 8])
                nc.vector.tensor_tensor(OUTv[:, :, :ne, :], F1be, F2Rv, AL.mult)

                if nd:
                    F2n = sm.tile([128, nd * N], BF16, tag="F2n")
                    F2nv = F2n[:].rearrange("p (a n) -> p a n", a=nd)
                    act(F2nv, W2v[:, ne:, :], AF.Exp, bias=b_ln2, scale=-1.0)
                    F1bd = F1v[:, :, None, :].to_broadcast([128, N, nd, 8])
                    F2nb = F2nv.rearrange("p a n -> p n a")[:, :, :, None] \
                        .to_broadcast([128, N, nd, 8])
                    nc.vector.tensor_tensor(OUTv[:, :, ne:, :], F1bd, F2nb, AL.mult)

                nc.sync.dma_start(
                    out[base: base + TILE_PAIRS, :].rearrange("(p n) f -> p (n f)", p=128),
                    OUT[:],
                )
            phase_barrier()

        if skew:
            # P1(0) P2(0) P1(1) P3(0) P2(1) P1(2) P3(1) ... P3(NG-1)
            phase1(0)
            phase2(0)
            for g in range(1, ngroups):
                phase1(g)
                phase3(g - 1)
                phase2(g)
            phase3(ngroups - 1)
        else:
            for g in range(ngroups):
                phase1(g)
                phase2(g)
                phase3(g)

    # Restrict activation-table membership so each phase's functions resolve
    # to one set (avoids the greedy first-set binding thrashing table loads).
    import concourse.bacc as bacc_mod
    from concourse.hw_specs import get_activation_tables as _real_gat
    keep = {"sqrt_and_others", "trig_and_small", "natural_log_exp_and_others"}

    def _gat(arch):
        return {k: (v if k in keep else set()) for k, v in _real_gat(arch).items()}

    bacc_mod.get_activation_tables = _gat
    try:
        nc.compile()
    finally:
        bacc_mod.get_activation_tables = _real_gat
    return nc


def _cst_arrays(N=N):
    zc = (0.75 * np.pi - 0.5 * SHFZ).astype(np.float32)
    cst = np.broadcast_to(zc, (128, 8)).copy()
    a2 = np.repeat((2.0 * SQ2 * SHFA).astype(np.float16), N)   # [a, n] layout
    cst16 = np.broadcast_to(a2, (128, 8 * N)).copy()
    return cst, cst16


def _run(vectors12: np.ndarray, trace: bool = False):
    if "nc" not in _CACHE:
        _CACHE["nc"] = _build_nc()
    nc = _CACHE["nc"]

    v = np.ascontiguousarray(np.asarray(vectors12, dtype=np.float32))
    pad = np.zeros((2, P_PAD - P_CORE, 3), np.float32)
    pad[:, :, 0] = 1.0  # unit vectors: all downstream math well-defined
    cst, cst16 = _cst_arrays()

    in_maps = []
    for i in range(NCORES):
        shard = v[:, i * P_CORE: (i + 1) * P_CORE, :]
        shard = np.concatenate([shard, pad], axis=1)
        in_maps.append({"vectors12": np.ascontiguousarray(shard),
                        "cst": cst, "cst16": cst16})

    res = run_bass_kernel_spmd(nc, in_maps, core_ids=list(range(NCORES)),
                               trace=trace)
    out = np.empty((P_TOTAL, 64), np.float32)
    for i in range(NCORES):
        shard_out = np.asarray(res.results[i]["out"])[:P_CORE]
        out[i * P_CORE: (i + 1) * P_CORE] = shard_out.astype(np.float32)
    return out, res


def kernel(vectors12, EtaA=None, Zeta=None, ShfA=None, ShfZ=None):
    out, _ = _run(vectors12, trace=False)
    return out
